# revision 9
# baseline (speedup 1.0000x reference)
"""Trainium2 Bass kernel for nn_DAC_structure (sparse dual-attention structure map).

For inputs q/k of shape (B*CH, L, H, E) = (64, 32, 8, 64):
  s  = softmax((q @ k^T) / sqrt(E))            per (batch-channel, head)
  m  = mean over the CH=8 channel group        -> [b, H, 32, 32]
  out_ps = element-repeat(m_ps, 32, 32)        -> [b, H, 1024, 1024]
  out_pn = tile(m_pn, 32, 32)                  -> [b, H, 1024, 1024]

Sharding: data-parallel over the true batch dim b = 8; core i handles batch i
(channel rows 8i..8i+8). No cross-device comms. Each core writes its own
[8, 1024, 1024] x2 fp16 output shard (rel err ~1.1e-3 vs the 2e-2 gate); the
host stacks the shards and upcasts to f32 during the gather.

The kernel is HBM-write-bound: per core the 16 SDMA engines sustain
~385 GB/s at 2 KB packets, ~401 at 4 KB, ~416 at 8 KB, so the 33.55 MB fp16
output floors at ~80 us of streaming. The program is shaped to (a) reach the
first output byte as early as possible and (b) use big DMA packets:
  - Inputs are marshalled ON THE HOST during sharding: cast to fp16 and
    pre-permuted into the exact [128 = (h%2)*64+e, 1024 = (chalf*4+h//2)*128
    + (c%4)*32 + l] transposed SBUF layout the QK^T matmuls consume. This
    removes the on-device f32->f16 casts and all 16 PE transposes from the
    serial path to the first write, and halves the input DMA bytes.
  - Input DMA is HBM-READ-latency bound (~200-450 ns per packet regardless
    of 1-4 KB size), so the q/k pair of each kind loads as ONE [128, 2048]
    DMA (4 KB lines, 128 packets) and everything rides the sync ring in
    series: ps pair first (gates the first write), pn pair + rep4 behind
    (they drain during the g0 compute window without stealing engine slots
    from the ps flight).
  - The pn partition-replication matrices (R4j permuted identities) are
    host-built constants DMA'd in, so no identity/iota build on device.
  - The ps-g0 block (matmuls, softmax, expand, write trigger) is emitted
    under tc.high_priority() so the Tile scheduler cannot interleave pn/g1
    work into its engine streams (that cost ~2.5 us of DVE bubbles in v2).
    The pn and g1 softmax chains run on the otherwise-idle GpSimd engine
    (except the tiny reciprocals - DVE only), PSUM->SBUF rep copies on
    ACT + GpSimd, so DVE belongs to the ps-g0 chain alone.
  - First write (ps g0) sources a [128, 2048] tile built by DVE and GpSimd
    in parallel (two [128,1024] halves, no extra serial latency) -> 4 KB
    packets. ps g1 uses [128, 8192] (16 KB packets), built off-path.
  - pn heads use rep4 tiles [128, 4096] (partition p holds head rows
    4p..4p+3, re-read 2x via a stride-0 mid dim) -> 8 KB packets, one 2 MB
    DMA per head. Replication runs on the PE (R4j matmuls).
  - All output DMAs use exactly 128 source partitions (HWDGE runs
    non-128-partition transfers at a fraction of the pace).
  - Stream order [ps g0, pn h0-3, pn h4-7, ps g1]; all later tiles build
    under the stream so the sync queue never starves.
"""

import sys

if "/opt/trn_rl_repo" not in sys.path:
    sys.path.insert(0, "/opt/trn_rl_repo")

from contextlib import ExitStack

import numpy as np

import concourse.bacc as bacc
import concourse.bass as bass
import concourse.mybir as mybir
import concourse.tile as tile

F32 = mybir.dt.float32
F16 = mybir.dt.float16

CH = 8   # channels per true batch
L = 32   # patch_num (seq len of the small attention)
H = 8    # heads
E = 64   # head dim
WIN = 1024
N_CORES = 8


def _host_tr_layout(x):
    """[8c, 32l, 8h, 64e] f32 -> [128, 1024] f16 in the transposed layout
    tr[(h%2)*64 + e, (c//4*4 + h//2)*128 + (c%4)*32 + l]."""
    x = np.asarray(x, dtype=np.float16)
    # [chalf, clo, l, hpair, hpar, e] -> [hpar, e, chalf, hpair, clo, l]
    x = x.reshape(2, 4, L, 4, 2, 64).transpose(4, 5, 0, 3, 1, 2)
    return x.reshape(128, 1024)


def _host_rep4():
    """[128, 512] f16: 4 permuted identities side by side.
    rep4[:, j*128:(j+1)*128][32*b + k, p] = (k == (4p+j) % 32)."""
    out = np.zeros((128, 512), dtype=np.float16)
    p = np.arange(128)
    for j in range(4):
        small = np.zeros((32, 128), dtype=np.float16)
        small[(4 * p + j) % 32, p] = 1.0
        out[:, j * 128:(j + 1) * 128] = np.tile(small, (4, 1))
    return out


def _group_mean_softmax(nc, pool, psum_s, qt, kt, g, kind):
    """QK^T matmuls + softmax + channel mean for h-group g -> M [128, 32].
    The ex/r/w/wx scratch tiles use ONE shared tag across all four groups
    (pool bufs=1), so each group's chain has a real WAW/WAR dependency on
    the previous group's - the Tile scheduler then CANNOT statically
    interleave a later group's DVE ops into the latency-critical ps-g0
    chain (observed to cost ~2.3 us of bubbles otherwise)."""
    ve = nc.vector
    s_ps = psum_s.tile([128, 256], F32, tag="spsum", name="spsum")
    for c in range(CH):
        chalf, clo = divmod(c, 4)
        for hh in range(4):
            h = g * 4 + hh
            col = (chalf * 4 + h // 2) * 128 + clo * 32
            prow = (h % 2) * 64
            nc.tensor.matmul(
                s_ps[hh * 32 : hh * 32 + 32, c * 32 : c * 32 + 32],
                qt[prow : prow + 64, col : col + 32],
                kt[prow : prow + 64, col : col + 32],
                start=True, stop=True,
                tile_position=(prow, hh * 32),
            )
    # exp output in fp16: halves ACT+DVE time on the serial path to the
    # first output DMA; the row-sum still accumulates in f32
    ex = pool.tile([128, 256], F16, tag="ex_sm", name=f"ex_{kind}")
    nc.scalar.activation(ex, s_ps, mybir.ActivationFunctionType.Exp, scale=1.0 / 8.0)
    r = pool.tile([128, 8], F32, tag="r_sm", name=f"r_{kind}")
    ex_cview = bass.AP(tensor=ex.tensor, offset=ex.offset,
                       ap=[list(ex.ap[0]), [32, 8], [1, 32]])
    ve.tensor_reduce(r, ex_cview, axis=mybir.AxisListType.X, op=mybir.AluOpType.add)
    w = pool.tile([128, 8], F32, tag="w_sm", name=f"w_{kind}")
    nc.vector.reciprocal(w, r)
    # post-reciprocal stages in fp16: values are softmax terms <= 1/CH, and
    # 16-bit doubles DVE throughput on this serial critical path
    wx = pool.tile([128, 256], F16, tag="wx_sm", name=f"wx_{kind}")
    ex_scl = bass.AP(tensor=ex.tensor, offset=ex.offset,
                     ap=[list(ex.ap[0]), [1, 32], [32, 8]])
    w_bc = bass.AP(tensor=w.tensor, offset=w.offset,
                   ap=[list(w.ap[0]), [0, 32], [1, 8]])
    wx_out = bass.AP(tensor=wx.tensor, offset=wx.offset,
                     ap=[list(wx.ap[0]), [8, 32], [1, 8]])
    ve.scalar_tensor_tensor(out=wx_out, in0=ex_scl, scalar=1.0 / CH, in1=w_bc,
                            op0=mybir.AluOpType.mult, op1=mybir.AluOpType.mult)
    m = pool.tile([128, 32], F16, tag=f"m_{kind}_{g}", name=f"m_{kind}_{g}")
    wx_in = bass.AP(tensor=wx.tensor, offset=wx.offset,
                    ap=[list(wx.ap[0]), [8, 32], [1, 8]])
    with nc.allow_low_precision(reason="8-term mean of softmax probs <= 1/8; "
                                "fp16 accum err ~1e-3 vs the 2e-2 gate"):
        ve.tensor_reduce(m, wx_in, axis=mybir.AxisListType.X,
                         op=mybir.AluOpType.add)
    return m


def _ps_expand_src(m):
    return bass.AP(tensor=m.tensor, offset=m.offset,
                   ap=[list(m.ap[0]), [1, 32], [0, 32]])


def _build_ps_tile_g0(nc, pool, m):
    """[128, 1024] (2 KB lines): ONE DVE copy so the first write triggers at
    the earliest possible moment (a second block would add either ~0.7 us of
    serial DVE time or an ACT/GpSimd dependency that schedules late; the 2
    vs 4 KB packet-rate delta over 8.4 MB is only ~0.45 us)."""
    exp_t = pool.tile([128, 1024], F16, tag="expand_ps_0", name="expand_ps_0")
    nc.vector.tensor_copy(exp_t, _ps_expand_src(m))
    return exp_t


def _build_ps_tile_g1(nc, pool, m):
    """[128, 4096] (8 KB lines; 16 KB lines measured no faster per byte and
    correlate with a 14-17 us engine-79 straggler tail), built off the
    critical path with doubling copies on DVE."""
    exp_t = pool.tile([128, 4096], F16, tag="expand_ps_1", name="expand_ps_1")
    nc.vector.tensor_copy(exp_t[:, 0:1024], _ps_expand_src(m))
    nc.vector.tensor_copy(exp_t[:, 1024:2048], exp_t[:, 0:1024])
    nc.vector.tensor_copy(exp_t[:, 2048:4096], exp_t[:, 0:2048])
    return exp_t


def _build_pn_rep4(nc, pool, psum_r, repm, m, g):
    """Tile expansion [128, 1024] f16 + per-head partition replication on
    the PE: rep4[p, j*1024 + c] = exp[hh*32 + (4p+j)%32, c] via matmuls with
    the R4j permuted identities, then PSUM->SBUF f16 copies split between
    ACT and DVE (GpSimd cannot read PSUM - BIR verifier rejects it).
    Each rep4 tile holds head rows 4p..4p+3 -> 8 KB DMA lines."""
    exp_t = pool.tile([128, 1024], F16, tag=f"expand_pn_{g}", name=f"expand_pn_{g}")
    src = bass.AP(tensor=m.tensor, offset=m.offset,
                  ap=[list(m.ap[0]), [0, 32], [1, 32]])
    nc.vector.tensor_copy(exp_t, src)
    reps = []
    for hh in range(4):
        rep = pool.tile([128, 4096], F16, tag=f"rep_pn_{g * 4 + hh}",
                        name=f"rep_pn_{g * 4 + hh}")
        for j in range(4):
            pr = psum_r.tile([128, 1024], F32, tag="rep_psum", name="rep_psum")
            for half in range(2):
                nc.tensor.matmul(
                    pr[:, half * 512 : (half + 1) * 512],
                    repm[hh * 32 : (hh + 1) * 32, j * 128 : (j + 1) * 128],
                    exp_t[hh * 32 : (hh + 1) * 32, half * 512 : (half + 1) * 512],
                    start=True, stop=True,
                    tile_position=(hh * 32, 0),
                )
            dst = rep[:, j * 1024 : (j + 1) * 1024]
            if j % 2 == 0:
                nc.scalar.activation(dst, pr, mybir.ActivationFunctionType.Copy)
            else:
                nc.vector.tensor_copy(dst, pr)
        reps.append(rep)
    return reps


def _write_ps_group(nc, exp_t, out_dram, g, cols, eng=None):
    """One 8 MB DMA per h-group: each source line re-read (stride-0 mid dim)
    -> identical consecutive output rows per partition; the HBM walk is
    fully sequential over the group's [4096, 1024] row span. The g0 write
    issues from the scalar ring: its DMA_DIRECT2D issues in ~420 ns there
    vs ~690 ns on sync, and it is the trigger that gates the whole stream."""
    pitch = exp_t.ap[0][0]
    n_lines = 32768 // cols
    src = bass.AP(tensor=exp_t.tensor, offset=exp_t.offset,
                  ap=[[pitch, 128], [0, n_lines], [1, cols]])
    dst = bass.AP(tensor=out_dram.tensor,
                  offset=out_dram.offset + g * 4 * WIN * WIN,
                  ap=[[32 * WIN, 128], [cols, n_lines], [1, cols]])
    (eng or nc.sync).dma_start(out=dst, in_=src)


def _write_pn_head(nc, rep, out_dram, h):
    """One 2 MB DMA per head: the [128, 4096] rep4 tile (512 rows) re-read
    2x via a stride-0 mid src dim. Partition p writes rows 4p..4p+3 of each
    512-row band as one contiguous 8 KB line."""
    pitch = rep.ap[0][0]
    src = bass.AP(tensor=rep.tensor, offset=rep.offset,
                  ap=[[pitch, 128], [0, 2], [1, 4096]])
    dst = bass.AP(tensor=out_dram.tensor,
                  offset=out_dram.offset + h * WIN * WIN,
                  ap=[[4 * WIN, 128], [512 * WIN, 2], [1, 4 * WIN]])
    nc.sync.dma_start(out=dst, in_=src)


def build_program():
    """Build and compile the per-core Bass program. Returns the Bacc object."""
    nc = bacc.Bacc(
        "TRN2",
        target_bir_lowering=False,
        debug=False,
        enable_asserts=False,
        num_devices=N_CORES,
    )
    ins = {}
    for name in ("qps_t", "kps_t", "qpn_t", "kpn_t"):
        ins[name] = nc.dram_tensor(name, [128, 1024], F16, kind="ExternalInput").ap()
    ins["rep4"] = nc.dram_tensor("rep4", [128, 512], F16, kind="ExternalInput").ap()
    out_ps = nc.dram_tensor("out_ps", [H, WIN, WIN], F16, kind="ExternalOutput").ap()
    out_pn = nc.dram_tensor("out_pn", [H, WIN, WIN], F16, kind="ExternalOutput").ap()

    with tile.TileContext(nc) as tc:
        with ExitStack() as ctx:
            pool = ctx.enter_context(tc.tile_pool(name="sbuf", bufs=1))
            chunk_pool = ctx.enter_context(tc.tile_pool(name="chunks", bufs=1))
            psum_s = ctx.enter_context(tc.tile_pool(name="spsum", bufs=2, space="PSUM"))
            psum_r = ctx.enter_context(tc.tile_pool(name="rpsum", bufs=2, space="PSUM"))

            # Input loads: ps pair split across the two HWDGE rings (sync +
            # scalar) so both land ~simultaneously (measured faster than one
            # combined 4 KB-line DMA: every DMA completion is barriered on
            # the chronically slow queue-host engine 79, and its per-packet
            # cost scales with line size). pn pair + rep4 queue right
            # behind and drain during the g0 compute window.
            tr = {}
            for key, eng in (("qps_t", nc.sync), ("kps_t", nc.scalar),
                             ("qpn_t", nc.sync), ("kpn_t", nc.scalar)):
                t = pool.tile([128, 1024], F16, tag=key, name=key)
                eng.dma_start(out=t, in_=ins[key])
                tr[key] = t
            repm = pool.tile([128, 512], F16, tag="rep4", name="rep4")
            nc.scalar.dma_start(out=repm, in_=ins["rep4"])

            # group 0: ps first, pinned to the front of the schedule
            with tc.high_priority():
                m = _group_mean_softmax(nc, chunk_pool, psum_s, tr["qps_t"], tr["kps_t"], 0, "ps")
                exp_ps = _build_ps_tile_g0(nc, pool, m)
                _write_ps_group(nc, exp_ps, out_ps, 0, cols=1024, eng=nc.scalar)

            m = _group_mean_softmax(nc, chunk_pool, psum_s, tr["qpn_t"], tr["kpn_t"], 0, "pn")
            reps = _build_pn_rep4(nc, pool, psum_r, repm, m, 0)
            for hh in range(4):
                _write_pn_head(nc, reps[hh], out_pn, hh)

            # group 1: pn first, ps last
            m = _group_mean_softmax(nc, chunk_pool, psum_s, tr["qpn_t"], tr["kpn_t"], 1, "pn")
            reps = _build_pn_rep4(nc, pool, psum_r, repm, m, 1)
            for hh in range(4):
                _write_pn_head(nc, reps[hh], out_pn, 4 + hh)

            m = _group_mean_softmax(nc, chunk_pool, psum_s, tr["qps_t"], tr["kps_t"], 1, "ps")
            exp_ps = _build_ps_tile_g1(nc, pool, m)
            _write_ps_group(nc, exp_ps, out_ps, 1, cols=4096)
    nc.compile()
    return nc


_NC_CACHE = None


def _get_nc():
    global _NC_CACHE
    if _NC_CACHE is None:
        _NC_CACHE = build_program()
    return _NC_CACHE


def run_sharded(queries_patch_size, queries_patch_num, keys_patch_size, keys_patch_num,
                trace=False, tmpdir=None):
    """Run the SPMD kernel on 8 cores; returns (full_ps, full_pn[, results])."""
    from concourse.bass_utils import run_bass_kernel_spmd

    nc = _get_nc()
    qps = np.asarray(queries_patch_size, dtype=np.float32)
    qpn = np.asarray(queries_patch_num, dtype=np.float32)
    kps = np.asarray(keys_patch_size, dtype=np.float32)
    kpn = np.asarray(keys_patch_num, dtype=np.float32)
    rep4 = _host_rep4()

    in_maps = []
    for b in range(N_CORES):
        sl = slice(b * CH, (b + 1) * CH)
        in_maps.append({
            "qps_t": np.ascontiguousarray(_host_tr_layout(qps[sl])),
            "kps_t": np.ascontiguousarray(_host_tr_layout(kps[sl])),
            "qpn_t": np.ascontiguousarray(_host_tr_layout(qpn[sl])),
            "kpn_t": np.ascontiguousarray(_host_tr_layout(kpn[sl])),
            "rep4": rep4,
        })
    res = run_bass_kernel_spmd(nc, in_maps, core_ids=list(range(N_CORES)), trace=trace,
                               tmpdir=tmpdir)
    full_ps = np.stack([np.asarray(res.results[b]["out_ps"]).astype(np.float32)
                        for b in range(N_CORES)], axis=0)
    full_pn = np.stack([np.asarray(res.results[b]["out_pn"]).astype(np.float32)
                        for b in range(N_CORES)], axis=0)
    if trace:
        return full_ps, full_pn, res
    return full_ps, full_pn


def kernel(queries_patch_size, queries_patch_num, keys_patch_size, keys_patch_num,
           values=None, patch_index=0, attn_mask=None):
    """Full-input entry point: takes the unsharded inputs, returns full outputs."""
    full_ps, full_pn = run_sharded(
        queries_patch_size, queries_patch_num, keys_patch_size, keys_patch_num
    )
    return full_ps, full_pn


# revision 10
# speedup vs baseline: 1.0235x; 1.0235x over previous
"""Trainium2 Bass kernel for nn_DAC_structure (sparse dual-attention structure map).

For inputs q/k of shape (B*CH, L, H, E) = (64, 32, 8, 64):
  s  = softmax((q @ k^T) / sqrt(E))            per (batch-channel, head)
  m  = mean over the CH=8 channel group        -> [b, H, 32, 32]
  out_ps = element-repeat(m_ps, 32, 32)        -> [b, H, 1024, 1024]
  out_pn = tile(m_pn, 32, 32)                  -> [b, H, 1024, 1024]

Sharding: data-parallel over the true batch dim b = 8; core i handles batch i
(channel rows 8i..8i+8). No cross-device comms. Each core writes its own
[8, 1024, 1024] x2 fp16 output shard (rel err ~1.1e-3 vs the 2e-2 gate); the
host stacks the shards and upcasts to f32 during the gather.

The kernel is HBM-write-bound: per core the 16 SDMA engines sustain
~385 GB/s at 2 KB packets, ~401 at 4 KB, ~416 at 8 KB, so the 33.55 MB fp16
output floors at ~80 us of streaming. The program is shaped to (a) reach the
first output byte as early as possible and (b) use big DMA packets:
  - Inputs are marshalled ON THE HOST during sharding: cast to fp16 and
    pre-permuted into the exact [128 = (h%2)*64+e, 1024 = (chalf*4+h//2)*128
    + (c%4)*32 + l] transposed SBUF layout the QK^T matmuls consume. This
    removes the on-device f32->f16 casts and all 16 PE transposes from the
    serial path to the first write, and halves the input DMA bytes.
  - Input DMA is HBM-READ-latency bound (~200-450 ns per packet regardless
    of 1-4 KB size), so the q/k pair of each kind loads as ONE [128, 2048]
    DMA (4 KB lines, 128 packets) and everything rides the sync ring in
    series: ps pair first (gates the first write), pn pair + rep4 behind
    (they drain during the g0 compute window without stealing engine slots
    from the ps flight).
  - The pn partition-replication matrices (R4j permuted identities) are
    host-built constants DMA'd in, so no identity/iota build on device.
  - The ps-g0 block (matmuls, softmax, expand, write trigger) is emitted
    under tc.high_priority() so the Tile scheduler cannot interleave pn/g1
    work into its engine streams (that cost ~2.5 us of DVE bubbles in v2).
    The pn and g1 softmax chains run on the otherwise-idle GpSimd engine
    (except the tiny reciprocals - DVE only), PSUM->SBUF rep copies on
    ACT + GpSimd, so DVE belongs to the ps-g0 chain alone.
  - First write (ps g0) sources a [128, 2048] tile built by DVE and GpSimd
    in parallel (two [128,1024] halves, no extra serial latency) -> 4 KB
    packets. ps g1 uses [128, 8192] (16 KB packets), built off-path.
  - pn heads use rep4 tiles [128, 4096] (partition p holds head rows
    4p..4p+3, re-read 2x via a stride-0 mid dim) -> 8 KB packets, one 2 MB
    DMA per head. Replication runs on the PE (R4j matmuls).
  - All output DMAs use exactly 128 source partitions (HWDGE runs
    non-128-partition transfers at a fraction of the pace).
  - Stream order [ps g0, pn h0-3, pn h4-7, ps g1]; all later tiles build
    under the stream so the sync queue never starves.
"""

import sys

if "/opt/trn_rl_repo" not in sys.path:
    sys.path.insert(0, "/opt/trn_rl_repo")

from contextlib import ExitStack

import numpy as np

import concourse.bacc as bacc
import concourse.bass as bass
import concourse.mybir as mybir
import concourse.tile as tile

F32 = mybir.dt.float32
F16 = mybir.dt.float16

CH = 8   # channels per true batch
L = 32   # patch_num (seq len of the small attention)
H = 8    # heads
E = 64   # head dim
WIN = 1024
N_CORES = 8


def _host_tr_layout(x):
    """[8c, 32l, 8h, 64e] f32 -> [128, 1024] f16 in the transposed layout
    tr[(h%2)*64 + e, (c//4*4 + h//2)*128 + (c%4)*32 + l]."""
    x = np.asarray(x, dtype=np.float16)
    # [chalf, clo, l, hpair, hpar, e] -> [hpar, e, chalf, hpair, clo, l]
    x = x.reshape(2, 4, L, 4, 2, 64).transpose(4, 5, 0, 3, 1, 2)
    return x.reshape(128, 1024)


def _host_rep4():
    """[128, 512] f16: 4 permuted identities side by side.
    rep4[:, j*128:(j+1)*128][32*b + k, p] = (k == (4p+j) % 32)."""
    out = np.zeros((128, 512), dtype=np.float16)
    p = np.arange(128)
    for j in range(4):
        small = np.zeros((32, 128), dtype=np.float16)
        small[(4 * p + j) % 32, p] = 1.0
        out[:, j * 128:(j + 1) * 128] = np.tile(small, (4, 1))
    return out


def _group_mean_softmax(nc, pool, psum_s, qt, kt, g, kind):
    """QK^T matmuls + softmax + channel mean for h-group g -> M [128, 32].
    The ex/r/w/wx scratch tiles use ONE shared tag across all four groups
    (pool bufs=1), so each group's chain has a real WAW/WAR dependency on
    the previous group's - the Tile scheduler then CANNOT statically
    interleave a later group's DVE ops into the latency-critical ps-g0
    chain (observed to cost ~2.3 us of bubbles otherwise)."""
    ve = nc.vector
    s_ps = psum_s.tile([128, 256], F32, tag="spsum", name="spsum")
    for c in range(CH):
        chalf, clo = divmod(c, 4)
        for hh in range(4):
            h = g * 4 + hh
            col = (chalf * 4 + h // 2) * 128 + clo * 32
            prow = (h % 2) * 64
            nc.tensor.matmul(
                s_ps[hh * 32 : hh * 32 + 32, c * 32 : c * 32 + 32],
                qt[prow : prow + 64, col : col + 32],
                kt[prow : prow + 64, col : col + 32],
                start=True, stop=True,
                tile_position=(prow, hh * 32),
            )
    # exp output in fp16: halves ACT+DVE time on the serial path to the
    # first output DMA; the row-sum still accumulates in f32
    ex = pool.tile([128, 256], F16, tag="ex_sm", name=f"ex_{kind}")
    nc.scalar.activation(ex, s_ps, mybir.ActivationFunctionType.Exp, scale=1.0 / 8.0)
    r = pool.tile([128, 8], F32, tag="r_sm", name=f"r_{kind}")
    ex_cview = bass.AP(tensor=ex.tensor, offset=ex.offset,
                       ap=[list(ex.ap[0]), [32, 8], [1, 32]])
    ve.tensor_reduce(r, ex_cview, axis=mybir.AxisListType.X, op=mybir.AluOpType.add)
    w = pool.tile([128, 8], F32, tag="w_sm", name=f"w_{kind}")
    nc.vector.reciprocal(w, r)
    # post-reciprocal stages in fp16: values are softmax terms <= 1/CH, and
    # 16-bit doubles DVE throughput on this serial critical path
    wx = pool.tile([128, 256], F16, tag="wx_sm", name=f"wx_{kind}")
    ex_scl = bass.AP(tensor=ex.tensor, offset=ex.offset,
                     ap=[list(ex.ap[0]), [1, 32], [32, 8]])
    w_bc = bass.AP(tensor=w.tensor, offset=w.offset,
                   ap=[list(w.ap[0]), [0, 32], [1, 8]])
    wx_out = bass.AP(tensor=wx.tensor, offset=wx.offset,
                     ap=[list(wx.ap[0]), [8, 32], [1, 8]])
    ve.scalar_tensor_tensor(out=wx_out, in0=ex_scl, scalar=1.0 / CH, in1=w_bc,
                            op0=mybir.AluOpType.mult, op1=mybir.AluOpType.mult)
    m = pool.tile([128, 32], F16, tag=f"m_{kind}_{g}", name=f"m_{kind}_{g}")
    wx_in = bass.AP(tensor=wx.tensor, offset=wx.offset,
                    ap=[list(wx.ap[0]), [8, 32], [1, 8]])
    with nc.allow_low_precision(reason="8-term mean of softmax probs <= 1/8; "
                                "fp16 accum err ~1e-3 vs the 2e-2 gate"):
        ve.tensor_reduce(m, wx_in, axis=mybir.AxisListType.X,
                         op=mybir.AluOpType.add)
    return m


def _ps_expand_src(m):
    return bass.AP(tensor=m.tensor, offset=m.offset,
                   ap=[list(m.ap[0]), [1, 32], [0, 32]])


def _build_ps_tile_g0(nc, pool, m):
    """[128, 1024] (2 KB lines): ONE DVE copy so the first write triggers at
    the earliest possible moment (a second block would add either ~0.7 us of
    serial DVE time or an ACT/GpSimd dependency that schedules late; the 2
    vs 4 KB packet-rate delta over 8.4 MB is only ~0.45 us)."""
    exp_t = pool.tile([128, 1024], F16, tag="expand_ps_0", name="expand_ps_0")
    nc.vector.tensor_copy(exp_t, _ps_expand_src(m))
    return exp_t


def _build_ps_tile_g1(nc, pool, m):
    """[128, 4096] (8 KB lines; 16 KB lines measured no faster per byte and
    correlate with a 14-17 us engine-79 straggler tail), built off the
    critical path with doubling copies on DVE."""
    exp_t = pool.tile([128, 4096], F16, tag="expand_ps_1", name="expand_ps_1")
    nc.vector.tensor_copy(exp_t[:, 0:1024], _ps_expand_src(m))
    nc.vector.tensor_copy(exp_t[:, 1024:2048], exp_t[:, 0:1024])
    nc.vector.tensor_copy(exp_t[:, 2048:4096], exp_t[:, 0:2048])
    return exp_t


def _build_pn_rep4(nc, pool, psum_r, repm, m, g):
    """Tile expansion [128, 1024] f16 + per-head partition replication on
    the PE: rep4[p, j*1024 + c] = exp[hh*32 + (4p+j)%32, c] via matmuls with
    the R4j permuted identities, then PSUM->SBUF f16 copies split between
    ACT and DVE (GpSimd cannot read PSUM - BIR verifier rejects it).
    Each rep4 tile holds head rows 4p..4p+3 -> 8 KB DMA lines."""
    exp_t = pool.tile([128, 1024], F16, tag=f"expand_pn_{g}", name=f"expand_pn_{g}")
    src = bass.AP(tensor=m.tensor, offset=m.offset,
                  ap=[list(m.ap[0]), [0, 32], [1, 32]])
    nc.vector.tensor_copy(exp_t, src)
    reps = []
    for hh in range(4):
        rep = pool.tile([128, 4096], F16, tag=f"rep_pn_{g * 4 + hh}",
                        name=f"rep_pn_{g * 4 + hh}")
        for j in range(4):
            pr = psum_r.tile([128, 1024], F32, tag="rep_psum", name="rep_psum")
            for half in range(2):
                nc.tensor.matmul(
                    pr[:, half * 512 : (half + 1) * 512],
                    repm[hh * 32 : (hh + 1) * 32, j * 128 : (j + 1) * 128],
                    exp_t[hh * 32 : (hh + 1) * 32, half * 512 : (half + 1) * 512],
                    start=True, stop=True,
                    tile_position=(hh * 32, 0),
                )
            dst = rep[:, j * 1024 : (j + 1) * 1024]
            if j % 2 == 0:
                nc.scalar.activation(dst, pr, mybir.ActivationFunctionType.Copy)
            else:
                nc.vector.tensor_copy(dst, pr)
        reps.append(rep)
    return reps


def _write_ps_group(nc, exp_t, out_dram, g, cols, eng=None):
    """One 8 MB DMA per h-group: each source line re-read (stride-0 mid dim)
    -> identical consecutive output rows per partition; the HBM walk is
    fully sequential over the group's [4096, 1024] row span. (Issuing the
    g0 trigger from the scalar ring looked cheaper on paper - ~420 vs ~690
    ns issue - but measured ~2 us slower end-to-end; all writes stay on
    sync.)"""
    pitch = exp_t.ap[0][0]
    n_lines = 32768 // cols
    src = bass.AP(tensor=exp_t.tensor, offset=exp_t.offset,
                  ap=[[pitch, 128], [0, n_lines], [1, cols]])
    dst = bass.AP(tensor=out_dram.tensor,
                  offset=out_dram.offset + g * 4 * WIN * WIN,
                  ap=[[32 * WIN, 128], [cols, n_lines], [1, cols]])
    (eng or nc.sync).dma_start(out=dst, in_=src)


def _write_pn_head(nc, rep, out_dram, h):
    """One 2 MB DMA per head: the [128, 4096] rep4 tile (512 rows) re-read
    2x via a stride-0 mid src dim. Partition p writes rows 4p..4p+3 of each
    512-row band as one contiguous 8 KB line."""
    pitch = rep.ap[0][0]
    src = bass.AP(tensor=rep.tensor, offset=rep.offset,
                  ap=[[pitch, 128], [0, 2], [1, 4096]])
    dst = bass.AP(tensor=out_dram.tensor,
                  offset=out_dram.offset + h * WIN * WIN,
                  ap=[[4 * WIN, 128], [512 * WIN, 2], [1, 4 * WIN]])
    nc.sync.dma_start(out=dst, in_=src)


def build_program():
    """Build and compile the per-core Bass program. Returns the Bacc object."""
    nc = bacc.Bacc(
        "TRN2",
        target_bir_lowering=False,
        debug=False,
        enable_asserts=False,
        num_devices=N_CORES,
    )
    ins = {}
    for name in ("qps_t", "kps_t", "qpn_t", "kpn_t"):
        ins[name] = nc.dram_tensor(name, [128, 1024], F16, kind="ExternalInput").ap()
    ins["rep4"] = nc.dram_tensor("rep4", [128, 512], F16, kind="ExternalInput").ap()
    out_ps = nc.dram_tensor("out_ps", [H, WIN, WIN], F16, kind="ExternalOutput").ap()
    out_pn = nc.dram_tensor("out_pn", [H, WIN, WIN], F16, kind="ExternalOutput").ap()

    with tile.TileContext(nc) as tc:
        with ExitStack() as ctx:
            pool = ctx.enter_context(tc.tile_pool(name="sbuf", bufs=1))
            chunk_pool = ctx.enter_context(tc.tile_pool(name="chunks", bufs=1))
            psum_s = ctx.enter_context(tc.tile_pool(name="spsum", bufs=2, space="PSUM"))
            psum_r = ctx.enter_context(tc.tile_pool(name="rpsum", bufs=2, space="PSUM"))

            # Input loads: ps pair split across the two HWDGE rings (sync +
            # scalar) so both land ~simultaneously (measured faster than one
            # combined 4 KB-line DMA: every DMA completion is barriered on
            # the chronically slow queue-host engine 79, and its per-packet
            # cost scales with line size). pn pair + rep4 queue right
            # behind and drain during the g0 compute window.
            tr = {}
            for key, eng in (("qps_t", nc.sync), ("kps_t", nc.scalar),
                             ("qpn_t", nc.sync), ("kpn_t", nc.scalar)):
                t = pool.tile([128, 1024], F16, tag=key, name=key)
                eng.dma_start(out=t, in_=ins[key])
                tr[key] = t
            repm = pool.tile([128, 512], F16, tag="rep4", name="rep4")
            nc.scalar.dma_start(out=repm, in_=ins["rep4"])

            # group 0: ps first, pinned to the front of the schedule
            with tc.high_priority():
                m = _group_mean_softmax(nc, chunk_pool, psum_s, tr["qps_t"], tr["kps_t"], 0, "ps")
                exp_ps = _build_ps_tile_g0(nc, pool, m)
                _write_ps_group(nc, exp_ps, out_ps, 0, cols=1024)

            m = _group_mean_softmax(nc, chunk_pool, psum_s, tr["qpn_t"], tr["kpn_t"], 0, "pn")
            reps = _build_pn_rep4(nc, pool, psum_r, repm, m, 0)
            for hh in range(4):
                _write_pn_head(nc, reps[hh], out_pn, hh)

            # group 1: pn first, ps last
            m = _group_mean_softmax(nc, chunk_pool, psum_s, tr["qpn_t"], tr["kpn_t"], 1, "pn")
            reps = _build_pn_rep4(nc, pool, psum_r, repm, m, 1)
            for hh in range(4):
                _write_pn_head(nc, reps[hh], out_pn, 4 + hh)

            m = _group_mean_softmax(nc, chunk_pool, psum_s, tr["qps_t"], tr["kps_t"], 1, "ps")
            exp_ps = _build_ps_tile_g1(nc, pool, m)
            _write_ps_group(nc, exp_ps, out_ps, 1, cols=4096)
    nc.compile()
    return nc


_NC_CACHE = None


def _get_nc():
    global _NC_CACHE
    if _NC_CACHE is None:
        _NC_CACHE = build_program()
    return _NC_CACHE


def run_sharded(queries_patch_size, queries_patch_num, keys_patch_size, keys_patch_num,
                trace=False, tmpdir=None):
    """Run the SPMD kernel on 8 cores; returns (full_ps, full_pn[, results])."""
    from concourse.bass_utils import run_bass_kernel_spmd

    nc = _get_nc()
    qps = np.asarray(queries_patch_size, dtype=np.float32)
    qpn = np.asarray(queries_patch_num, dtype=np.float32)
    kps = np.asarray(keys_patch_size, dtype=np.float32)
    kpn = np.asarray(keys_patch_num, dtype=np.float32)
    rep4 = _host_rep4()

    in_maps = []
    for b in range(N_CORES):
        sl = slice(b * CH, (b + 1) * CH)
        in_maps.append({
            "qps_t": np.ascontiguousarray(_host_tr_layout(qps[sl])),
            "kps_t": np.ascontiguousarray(_host_tr_layout(kps[sl])),
            "qpn_t": np.ascontiguousarray(_host_tr_layout(qpn[sl])),
            "kpn_t": np.ascontiguousarray(_host_tr_layout(kpn[sl])),
            "rep4": rep4,
        })
    res = run_bass_kernel_spmd(nc, in_maps, core_ids=list(range(N_CORES)), trace=trace,
                               tmpdir=tmpdir)
    full_ps = np.stack([np.asarray(res.results[b]["out_ps"]).astype(np.float32)
                        for b in range(N_CORES)], axis=0)
    full_pn = np.stack([np.asarray(res.results[b]["out_pn"]).astype(np.float32)
                        for b in range(N_CORES)], axis=0)
    if trace:
        return full_ps, full_pn, res
    return full_ps, full_pn


def kernel(queries_patch_size, queries_patch_num, keys_patch_size, keys_patch_num,
           values=None, patch_index=0, attn_mask=None):
    """Full-input entry point: takes the unsharded inputs, returns full outputs."""
    full_ps, full_pn = run_sharded(
        queries_patch_size, queries_patch_num, keys_patch_size, keys_patch_num
    )
    return full_ps, full_pn
